# revision 11
# baseline (speedup 1.0000x reference)
"""DWT (db4-style, depthwise stride-2, reflect-pad) layer as a Trainium2
Bass/Tile kernel — T-sharded, bf16 end-to-end.

Math: for input x[B, T, C] and 8-tap filters lo/hi the reference computes a
reflect-pad-7, stride-2, depthwise cross-correlation cropped by 3 per side:

    out[b, t', c]     = sum_k lo[k] * xe[2 t' + k][b, c]
    out[b, t', C + c] = sum_k hi[k] * xe[2 t' + k][b, c]

with xe[0] = x[1], xe[u] = x[u - 1] for u in [1, T+1), xe[T+1+j] = x[T-2-j],
and t' in [0, T/2 - 2).

Why this layout: the profile of the batch-sharded fp32 version showed the
kernel is DMA *packet-rate* bound (~77 ns per descriptor per engine, 16
engines), not byte bound — 2 KB load lines ran at 22 GB/s/engine and 1 KB
store lines at 15 GB/s/engine.  Sharding over T instead of B makes every
DMA line a full [B, C] row: 4 KB (bf16 in) / 8 KB (bf16 out) contiguous,
~8x fewer packets.  bf16 also halves HBM bytes (17 MB/core total, ~47 us
at the 358 GB/s per-core roofline) and runs the TensorE matmuls at full
rate (fp32 ran at 1/4 rate and was a co-bottleneck at 128 us).

Device mapping (per core, 1/8 of the t' axis = 1024 outputs + halo):
  - host builds the reflect-extended xe [T+8, B, C] in bf16 and ships slab
    xe[2048 i : 2048 i + 2056] transposed to [2056, B, C];
  - time on the SBUF partition axis, 2 steps per partition (polyphase):
    xt[p, (j b c)] = slab[u0 + 2p + j]; banded stationary matrices
    W_q[p, m] = f[2 (p - m) + j] (f in {lo, hi}, j in {0, 1}, p - m in
    [0, 4)) accumulate over j in PSUM;
  - the [B*C = 2048]-wide moving dim is processed in 4 chunks of 512
    (PSUM bank width), 2 filters x 2 j-phases x 4 chunks = 16 matmuls
    per tile, each [*, 125] x [*, 512];
  - 8 PSUM banks hold the 4 chunks x {lo, hi}; copies into the
    [t', (b, lo|hi, c)] SBUF output tile are split across the Vector,
    Scalar (activation copy) and GpSimd engines, casting fp32 -> bf16;
  - loads issue on the Sync HWDGE queue, stores on the Scalar HWDGE
    queue; each queue's descriptors are spread per-packet across all 16
    SDMA engines by hardware, so a single big dma_start per tile
    saturates them (verified in the trace).
  - 8 full tiles (125 outputs) + 1 rump tile (24 outputs, 27 partitions)
    per core; cores 0-6 produce 1024 valid t', core 7 produces 1022 and
    the host crops the global concat to 8190 rows.

Correctness margin: bf16 quantization of x and the filters gives a max
relative error of ~2e-3 vs the fp32 reference, 10x under the 2e-2 gate
(PSUM accumulates in fp32).
"""

import numpy as np
import ml_dtypes

import concourse.bacc as bacc
import concourse.mybir as mybir
import concourse.tile as tile
from concourse.bass_utils import run_bass_kernel_spmd

F32 = mybir.dt.float32
BF16 = mybir.dt.bfloat16
NP_BF16 = ml_dtypes.bfloat16

B, T, C = 32, 16384, 64
N_CORES = 8
TS = 1024            # t' outputs per core
SLAB = 2056          # input rows per core (incl halo + reflect)
M = 125              # outputs per full tile (banded W needs p - m in [0,4))
NF = B * C           # 2048 moving columns
CHUNK = 512          # matmul moving cols per PSUM bank (2 KB fp32)
NCH = NF // CHUNK    # 4
NT = 8               # full tiles per core
MR = TS - NT * M     # 24 rump outputs
PR = MR + 3          # 27 rump partitions


def _build_nc():
    nc = bacc.Bacc("TRN2", target_bir_lowering=False, debug=False)
    x_d = nc.dram_tensor("x", [SLAB, B, C], BF16, kind="ExternalInput")
    w_d = nc.dram_tensor("w", [128, 4 * M], BF16, kind="ExternalInput")
    # planar per-row layout [t', (lo|hi, b, c)]; host interleaves to [B,T',2C]
    o_d = nc.dram_tensor("out", [TS, 2 * B * C], BF16, kind="ExternalOutput")

    with tile.TileContext(nc) as tc:
        with (
            tc.tile_pool(name="wpool", bufs=1) as wpool,
            tc.tile_pool(name="xin", bufs=4) as xpool,
            tc.tile_pool(name="oout", bufs=4) as opool,
            tc.tile_pool(name="ps", bufs=8, space="PSUM") as pspool,
        ):
            w_t = wpool.tile([128, 4 * M], BF16)
            nc.scalar.dma_start(out=w_t[:], in_=w_d[:])
            # order: lo_j0, lo_j1, hi_j0, hi_j1
            w_ap = [w_t[:, q * M:(q + 1) * M] for q in range(4)]

            def cp(eng, out, in_):
                if eng is nc.scalar:
                    eng.copy(out, in_)
                else:
                    eng.tensor_copy(out=out, in_=in_)

            # 8 copies per tile: 4 vector + 4 scalar (gpsimd cannot read PSUM)
            cp_eng = [nc.vector, nc.scalar, nc.vector, nc.scalar,
                      nc.scalar, nc.vector, nc.scalar, nc.vector]
            # stores are the bottleneck: SWDGE (gpsimd) feeds descriptors at
            # ~150 GB/s and HWDGE stores pin to a 5-engine subset (~125 GB/s)
            # -> split each tile's store rows across both paths
            SW_ROWS = 75

            for k in range(NT + 1):
                nm = M if k < NT else MR
                npart = 128 if k < NT else PR
                u0 = 250 * k
                xt = xpool.tile([128, 2 * NF], BF16, tag="xt")
                nc.scalar.dma_start(
                    out=xt[0:npart],
                    in_=x_d[u0:u0 + 2 * npart]
                    .rearrange("(p j) b c -> p (j b c)", j=2))
                xv = xt[:].rearrange("p (j w) -> p j w", j=2)
                ot = opool.tile([128, 2 * NF], BF16, tag="ot")
                ov = ot[:].rearrange("p (f w) -> p f w", f=2)
                # 8 PSUM banks: lo chunks 0-3, hi chunks 0-3
                ps = [pspool.tile([M, CHUNK], F32, tag="ps", name=f"ps{i}")
                      for i in range(8)]
                # filters outer, chunks inner: stationary weights loaded 4x
                # per tile instead of 16x; accumulation groups interleave
                # across banks (start q=2f, stop q=2f+1)
                for q in range(4):
                    f, j = q // 2, q % 2
                    for cc in range(NCH):
                        nc.tensor.matmul(
                            out=ps[4 * f + cc][0:nm],
                            lhsT=w_ap[q][0:npart, 0:nm],
                            rhs=xv[0:npart, j, CHUNK * cc:CHUNK * (cc + 1)],
                            start=(j == 0), stop=(j == 1),
                            skip_group_check=True)
                    if j == 1:
                        # filter f complete: drain its 4 banks (contiguous
                        # fp32 -> bf16 casts into the planar output half)
                        for cc in range(NCH):
                            eng = cp_eng[4 * f + cc]
                            out_ap = ov[0:nm, f, CHUNK * cc:CHUNK * (cc + 1)]
                            in_ap = ps[4 * f + cc][0:nm]
                            if eng is nc.scalar:
                                eng.copy(out_ap, in_ap)
                            else:
                                eng.tensor_copy(out=out_ap, in_=in_ap)
                sw = min(SW_ROWS, nm)
                nc.gpsimd.dma_start(out=o_d[M * k:M * k + sw], in_=ot[0:sw])
                if nm > sw:
                    nc.sync.dma_start(out=o_d[M * k + sw:M * k + nm],
                                      in_=ot[sw:nm])

    nc.compile()
    return nc


def _build_w(dec_lo: np.ndarray, dec_hi: np.ndarray) -> np.ndarray:
    """Banded stationary matrices [128, 4*M] bf16: order lo_j0, lo_j1, hi_j0, hi_j1."""
    lo = np.asarray(dec_lo, np.float32)
    hi = np.asarray(dec_hi, np.float32)
    w = np.zeros((128, 4, M), np.float32)
    for m in range(M):
        for d in range(4):
            w[m + d, 0, m] = lo[2 * d]
            w[m + d, 1, m] = lo[2 * d + 1]
            w[m + d, 2, m] = hi[2 * d]
            w[m + d, 3, m] = hi[2 * d + 1]
    return w.reshape(128, 4 * M).astype(NP_BF16)


_NC_CACHE = {}


def _get_nc():
    if "nc" not in _NC_CACHE:
        _NC_CACHE["nc"] = _build_nc()
    return _NC_CACHE["nc"]


def prepare_in_maps(x, dec_lo, dec_hi):
    xb = np.asarray(x).astype(NP_BF16)          # [B, T, C]
    head = xb[:, 1:2]
    tails = [xb[:, T - 2 - j:T - 1 - j] for j in range(5)]
    z = np.zeros((B, 2, C), NP_BF16)
    xe = np.concatenate([head, xb] + tails + [z], axis=1)   # [B, T+8, C]
    w = _build_w(dec_lo, dec_hi)
    return [
        {"x": np.ascontiguousarray(
            xe[:, 2048 * i:2048 * i + SLAB].transpose(1, 0, 2)),
         "w": w}
        for i in range(N_CORES)
    ]


def kernel(x: np.ndarray, dec_lo: np.ndarray, dec_hi: np.ndarray) -> np.ndarray:
    x = np.asarray(x, np.float32)
    assert x.shape == (B, T, C), x.shape
    nc = _get_nc()
    in_maps = prepare_in_maps(x, dec_lo, dec_hi)
    res = run_bass_kernel_spmd(nc, in_maps, core_ids=list(range(N_CORES)))
    # device outputs are [TS, (f b c)] t'-chunks -> concat, crop, interleave
    # to [B, T', 2C] (approx channels then detail channels)
    out = np.concatenate(
        [res.results[i]["out"].reshape(TS, 2, B, C) for i in range(N_CORES)],
        axis=0)
    out = out[:T // 2 - 2].astype(np.float32)        # [T', 2, B, C]
    return np.ascontiguousarray(
        out.transpose(2, 0, 1, 3).reshape(B, T // 2 - 2, 2 * C))


# revision 13
# speedup vs baseline: 1.1678x; 1.1678x over previous
"""DWT (db4-style, depthwise stride-2, reflect-pad) layer as a Trainium2
Bass/Tile kernel — T-sharded, bf16 end-to-end.

Math: for input x[B, T, C] and 8-tap filters lo/hi the reference computes a
reflect-pad-7, stride-2, depthwise cross-correlation cropped by 3 per side:

    out[b, t', c]     = sum_k lo[k] * xe[2 t' + k][b, c]
    out[b, t', C + c] = sum_k hi[k] * xe[2 t' + k][b, c]

with xe[0] = x[1], xe[u] = x[u - 1] for u in [1, T+1), xe[T+1+j] = x[T-2-j],
and t' in [0, T/2 - 2).

Why this layout: the profile of the batch-sharded fp32 version showed the
kernel is DMA *packet-rate* bound (~77 ns per descriptor per engine, 16
engines), not byte bound — 2 KB load lines ran at 22 GB/s/engine and 1 KB
store lines at 15 GB/s/engine.  Sharding over T instead of B makes every
DMA line a full [B, C] row: 4 KB (bf16 in) / 8 KB (bf16 out) contiguous,
~8x fewer packets.  bf16 also halves HBM bytes (17 MB/core total, ~47 us
at the 358 GB/s per-core roofline) and runs the TensorE matmuls at full
rate (fp32 ran at 1/4 rate and was a co-bottleneck at 128 us).

Device mapping (per core, 1/8 of the t' axis = 1024 outputs + halo):
  - host builds the reflect-extended xe [T+8, B, C] in bf16 and ships slab
    xe[2048 i : 2048 i + 2056] transposed to [2056, B, C];
  - time on the SBUF partition axis, 2 steps per partition (polyphase):
    xt[p, (j b c)] = slab[u0 + 2p + j]; banded stationary matrices
    W_q[p, m] = f[2 (p - m) + j] (f in {lo, hi}, j in {0, 1}, p - m in
    [0, 4)) accumulate over j in PSUM;
  - the [B*C = 2048]-wide moving dim is processed in 4 chunks of 512
    (PSUM bank width), 2 filters x 2 j-phases x 4 chunks = 16 matmuls
    per tile, each [*, 125] x [*, 512];
  - 8 PSUM banks hold the 4 chunks x {lo, hi}; copies into the
    [t', (b, lo|hi, c)] SBUF output tile are split across the Vector,
    Scalar (activation copy) and GpSimd engines, casting fp32 -> bf16;
  - loads issue on the Sync HWDGE queue, stores on the Scalar HWDGE
    queue; each queue's descriptors are spread per-packet across all 16
    SDMA engines by hardware, so a single big dma_start per tile
    saturates them (verified in the trace).
  - 8 full tiles (125 outputs) + 1 rump tile (24 outputs, 27 partitions)
    per core; cores 0-6 produce 1024 valid t', core 7 produces 1022 and
    the host crops the global concat to 8190 rows.

Correctness margin: bf16 quantization of x and the filters gives a max
relative error of ~2e-3 vs the fp32 reference, 10x under the 2e-2 gate
(PSUM accumulates in fp32).
"""

import numpy as np
import ml_dtypes

import concourse.bacc as bacc
import concourse.mybir as mybir
import concourse.tile as tile
from concourse.bass_utils import run_bass_kernel_spmd

F32 = mybir.dt.float32
BF16 = mybir.dt.bfloat16
NP_BF16 = ml_dtypes.bfloat16

B, T, C = 32, 16384, 64
N_CORES = 8
TS = 1024            # t' outputs per core
SLAB = 2056          # input rows per core (incl halo + reflect)
M = 125              # outputs per full tile (banded W needs p - m in [0,4))
NF = B * C           # 2048 moving columns
CHUNK = 512          # matmul moving cols per PSUM bank (2 KB fp32)
NCH = NF // CHUNK    # 4
NT = 8               # full tiles per core
MR = TS - NT * M     # 24 rump outputs
PR = MR + 3          # 27 rump partitions


def _build_nc():
    nc = bacc.Bacc("TRN2", target_bir_lowering=False, debug=False)
    x_d = nc.dram_tensor("x", [SLAB, B, C], BF16, kind="ExternalInput")
    w_d = nc.dram_tensor("w", [128, 4 * M], BF16, kind="ExternalInput")
    # planar per-row layout [t', (lo|hi, b, c)]; host interleaves to [B,T',2C]
    o_d = nc.dram_tensor("out", [TS, 2 * B * C], BF16, kind="ExternalOutput")

    with tile.TileContext(nc) as tc:
        with (
            tc.tile_pool(name="wpool", bufs=1) as wpool,
            tc.tile_pool(name="xin", bufs=NT + 1) as xpool,
            tc.tile_pool(name="oout", bufs=4) as opool,
            tc.tile_pool(name="ps", bufs=8, space="PSUM") as pspool,
        ):
            w_t = wpool.tile([128, 4 * M], BF16)
            nc.scalar.dma_start(out=w_t[:], in_=w_d[:])

            # issue ALL loads upfront on the sync HWDGE queue: they saturate
            # all 16 SDMA engines (~390 GB/s) with nothing queued ahead of
            # them; SBUF easily holds all 9 input tiles (72 KB/partition)
            xts = []
            for k in range(NT + 1):
                npart = 128 if k < NT else PR
                xt = xpool.tile([128, 2 * NF], BF16, tag="xt", name=f"xt{k}")
                nc.sync.dma_start(
                    out=xt[0:npart],
                    in_=x_d[250 * k:250 * k + 2 * npart]
                    .rearrange("(p j) b c -> p (j b c)", j=2))
                xts.append(xt)
            # order: lo_j0, lo_j1, hi_j0, hi_j1
            w_ap = [w_t[:, q * M:(q + 1) * M] for q in range(4)]

            def cp(eng, out, in_):
                if eng is nc.scalar:
                    eng.copy(out, in_)
                else:
                    eng.tensor_copy(out=out, in_=in_)

            # 8 copies per tile: 4 vector + 4 scalar (gpsimd cannot read PSUM)
            cp_eng = [nc.vector, nc.scalar, nc.vector, nc.scalar,
                      nc.scalar, nc.vector, nc.scalar, nc.vector]
            # stores are the bottleneck: SWDGE (gpsimd) feeds ~150 GB/s and
            # sync-queue HWDGE stores pin to a 10-engine subset; split each
            # tile's store rows across both paths to balance engine bytes
            SW_ROWS = 100

            for k in range(NT + 1):
                nm = M if k < NT else MR
                npart = 128 if k < NT else PR
                xv = xts[k][:].rearrange("p (j w) -> p j w", j=2)
                ot = opool.tile([128, 2 * NF], BF16, tag="ot")
                ov = ot[:].rearrange("p (f w) -> p f w", f=2)
                # 8 PSUM banks: lo chunks 0-3, hi chunks 0-3
                ps = [pspool.tile([M, CHUNK], F32, tag="ps", name=f"ps{i}")
                      for i in range(8)]
                # filters outer, chunks inner: stationary weights loaded 4x
                # per tile instead of 16x; accumulation groups interleave
                # across banks (start q=2f, stop q=2f+1)
                for q in range(4):
                    f, j = q // 2, q % 2
                    for cc in range(NCH):
                        nc.tensor.matmul(
                            out=ps[4 * f + cc][0:nm],
                            lhsT=w_ap[q][0:npart, 0:nm],
                            rhs=xv[0:npart, j, CHUNK * cc:CHUNK * (cc + 1)],
                            start=(j == 0), stop=(j == 1),
                            skip_group_check=True)
                    if j == 1:
                        # filter f complete: drain its 4 banks (contiguous
                        # fp32 -> bf16 casts into the planar output half)
                        for cc in range(NCH):
                            eng = cp_eng[4 * f + cc]
                            out_ap = ov[0:nm, f, CHUNK * cc:CHUNK * (cc + 1)]
                            in_ap = ps[4 * f + cc][0:nm]
                            if eng is nc.scalar:
                                eng.copy(out_ap, in_ap)
                            else:
                                eng.tensor_copy(out=out_ap, in_=in_ap)
                sw = min(SW_ROWS, nm)
                nc.gpsimd.dma_start(out=o_d[M * k:M * k + sw], in_=ot[0:sw])
                if nm > sw:
                    nc.sync.dma_start(out=o_d[M * k + sw:M * k + nm],
                                      in_=ot[sw:nm])

    nc.compile()
    return nc


def _build_w(dec_lo: np.ndarray, dec_hi: np.ndarray) -> np.ndarray:
    """Banded stationary matrices [128, 4*M] bf16: order lo_j0, lo_j1, hi_j0, hi_j1."""
    lo = np.asarray(dec_lo, np.float32)
    hi = np.asarray(dec_hi, np.float32)
    w = np.zeros((128, 4, M), np.float32)
    for m in range(M):
        for d in range(4):
            w[m + d, 0, m] = lo[2 * d]
            w[m + d, 1, m] = lo[2 * d + 1]
            w[m + d, 2, m] = hi[2 * d]
            w[m + d, 3, m] = hi[2 * d + 1]
    return w.reshape(128, 4 * M).astype(NP_BF16)


_NC_CACHE = {}


def _get_nc():
    if "nc" not in _NC_CACHE:
        _NC_CACHE["nc"] = _build_nc()
    return _NC_CACHE["nc"]


def prepare_in_maps(x, dec_lo, dec_hi):
    xb = np.asarray(x).astype(NP_BF16)          # [B, T, C]
    head = xb[:, 1:2]
    tails = [xb[:, T - 2 - j:T - 1 - j] for j in range(5)]
    z = np.zeros((B, 2, C), NP_BF16)
    xe = np.concatenate([head, xb] + tails + [z], axis=1)   # [B, T+8, C]
    w = _build_w(dec_lo, dec_hi)
    return [
        {"x": np.ascontiguousarray(
            xe[:, 2048 * i:2048 * i + SLAB].transpose(1, 0, 2)),
         "w": w}
        for i in range(N_CORES)
    ]


def kernel(x: np.ndarray, dec_lo: np.ndarray, dec_hi: np.ndarray) -> np.ndarray:
    x = np.asarray(x, np.float32)
    assert x.shape == (B, T, C), x.shape
    nc = _get_nc()
    in_maps = prepare_in_maps(x, dec_lo, dec_hi)
    res = run_bass_kernel_spmd(nc, in_maps, core_ids=list(range(N_CORES)))
    # device outputs are [TS, (f b c)] t'-chunks -> concat, crop, interleave
    # to [B, T', 2C] (approx channels then detail channels)
    out = np.concatenate(
        [res.results[i]["out"].reshape(TS, 2, B, C) for i in range(N_CORES)],
        axis=0)
    out = out[:T // 2 - 2].astype(np.float32)        # [T', 2, B, C]
    return np.ascontiguousarray(
        out.transpose(2, 0, 1, 3).reshape(B, T // 2 - 2, 2 * C))


# revision 15
# speedup vs baseline: 1.3080x; 1.1200x over previous
"""DWT (db4-style, depthwise stride-2, reflect-pad) layer as a Trainium2
Bass/Tile kernel — T-sharded, bf16 end-to-end.

Math: for input x[B, T, C] and 8-tap filters lo/hi the reference computes a
reflect-pad-7, stride-2, depthwise cross-correlation cropped by 3 per side:

    out[b, t', c]     = sum_k lo[k] * xe[2 t' + k][b, c]
    out[b, t', C + c] = sum_k hi[k] * xe[2 t' + k][b, c]

with xe[0] = x[1], xe[u] = x[u - 1] for u in [1, T+1), xe[T+1+j] = x[T-2-j],
and t' in [0, T/2 - 2).

Why this layout: the profile of the batch-sharded fp32 version showed the
kernel is DMA *packet-rate* bound (~77 ns per descriptor per engine, 16
engines), not byte bound — 2 KB load lines ran at 22 GB/s/engine and 1 KB
store lines at 15 GB/s/engine.  Sharding over T instead of B makes every
DMA line a full [B, C] row: 4 KB (bf16 in) / 8 KB (bf16 out) contiguous,
~8x fewer packets.  bf16 also halves HBM bytes (17 MB/core total, ~47 us
at the 358 GB/s per-core roofline) and runs the TensorE matmuls at full
rate (fp32 ran at 1/4 rate and was a co-bottleneck at 128 us).

Device mapping (per core, 1/8 of the t' axis = 1024 outputs + halo):
  - host builds the reflect-extended xe [T+8, B, C] in bf16 and ships slab
    xe[2048 i : 2048 i + 2056] transposed to [2056, B, C];
  - time on the SBUF partition axis, 2 steps per partition (polyphase):
    xt[p, (j b c)] = slab[u0 + 2p + j]; banded stationary matrices
    W_q[p, m] = f[2 (p - m) + j] (f in {lo, hi}, j in {0, 1}, p - m in
    [0, 4)) accumulate over j in PSUM;
  - the [B*C = 2048]-wide moving dim is processed in 4 chunks of 512
    (PSUM bank width), 2 filters x 2 j-phases x 4 chunks = 16 matmuls
    per tile, each [*, 125] x [*, 512];
  - 8 PSUM banks hold the 4 chunks x {lo, hi}; copies into the
    [t', (b, lo|hi, c)] SBUF output tile are split across the Vector,
    Scalar (activation copy) and GpSimd engines, casting fp32 -> bf16;
  - loads issue on the Sync HWDGE queue, stores on the Scalar HWDGE
    queue; each queue's descriptors are spread per-packet across all 16
    SDMA engines by hardware, so a single big dma_start per tile
    saturates them (verified in the trace).
  - 8 full tiles (125 outputs) + 1 rump tile (24 outputs, 27 partitions)
    per core; cores 0-6 produce 1024 valid t', core 7 produces 1022 and
    the host crops the global concat to 8190 rows.

Correctness margin: bf16 quantization of x and the filters gives a max
relative error of ~2e-3 vs the fp32 reference, 10x under the 2e-2 gate
(PSUM accumulates in fp32).
"""

import numpy as np
import ml_dtypes

import concourse.bacc as bacc
import concourse.mybir as mybir
import concourse.tile as tile
from concourse.bass_utils import run_bass_kernel_spmd

F32 = mybir.dt.float32
BF16 = mybir.dt.bfloat16
NP_BF16 = ml_dtypes.bfloat16

B, T, C = 32, 16384, 64
N_CORES = 8
TS = 1024            # t' outputs per core
SLAB = 2056          # input rows per core (incl halo + reflect)
M = 125              # outputs per full tile (banded W needs p - m in [0,4))
NF = B * C           # 2048 moving columns
CHUNK = 512          # matmul moving cols per PSUM bank (2 KB fp32)
NCH = NF // CHUNK    # 4
NT = 8               # full tiles per core
MR = TS - NT * M     # 24 rump outputs
PR = MR + 3          # 27 rump partitions


def _build_nc():
    nc = bacc.Bacc("TRN2", target_bir_lowering=False, debug=False)
    x_d = nc.dram_tensor("x", [SLAB, B, C], BF16, kind="ExternalInput")
    w_d = nc.dram_tensor("w", [128, 4 * M], BF16, kind="ExternalInput")
    # planar per-row layout [t', (lo|hi, b, c)]; host interleaves to [B,T',2C]
    o_d = nc.dram_tensor("out", [TS, 2 * B * C], BF16, kind="ExternalOutput")

    with tile.TileContext(nc) as tc:
        with (
            tc.tile_pool(name="wpool", bufs=1) as wpool,
            tc.tile_pool(name="xin", bufs=NT + 1) as xpool,
            tc.tile_pool(name="oout", bufs=4) as opool,
            tc.tile_pool(name="ps", bufs=8, space="PSUM") as pspool,
        ):
            w_t = wpool.tile([128, 4 * M], BF16)
            nc.sync.dma_start(out=w_t[:], in_=w_d[:])

            # issue ALL loads upfront on the sync HWDGE queue: they saturate
            # all 16 SDMA engines (~390 GB/s) with nothing queued ahead of
            # them; SBUF easily holds all 9 input tiles (72 KB/partition)
            xts = []
            for k in range(NT + 1):
                npart = 128 if k < NT else PR
                xt = xpool.tile([128, 2 * NF], BF16, tag="xt", name=f"xt{k}")
                nc.sync.dma_start(
                    out=xt[0:npart],
                    in_=x_d[250 * k:250 * k + 2 * npart]
                    .rearrange("(p j) b c -> p (j b c)", j=2))
                xts.append(xt)
            # order: lo_j0, lo_j1, hi_j0, hi_j1
            w_ap = [w_t[:, q * M:(q + 1) * M] for q in range(4)]

            def cp(eng, out, in_):
                if eng is nc.scalar:
                    eng.copy(out, in_)
                else:
                    eng.tensor_copy(out=out, in_=in_)

            # 8 copies per tile: 4 vector + 4 scalar (gpsimd cannot read PSUM)
            cp_eng = [nc.vector, nc.scalar, nc.vector, nc.scalar,
                      nc.scalar, nc.vector, nc.scalar, nc.vector]
            # stores are the bottleneck: SWDGE (gpsimd) feeds ~150 GB/s and
            # sync-queue HWDGE stores pin to a 10-engine subset; split each
            # tile's store rows across both paths to balance engine bytes
            SW_ROWS = 50

            for k in range(NT + 1):
                nm = M if k < NT else MR
                npart = 128 if k < NT else PR
                xv = xts[k][:].rearrange("p (j w) -> p j w", j=2)
                ot = opool.tile([128, 2 * NF], BF16, tag="ot")
                ov = ot[:].rearrange("p (f w) -> p f w", f=2)
                # 8 PSUM banks: lo chunks 0-3, hi chunks 0-3
                ps = [pspool.tile([M, CHUNK], F32, tag="ps", name=f"ps{i}")
                      for i in range(8)]
                # filters outer, chunks inner: stationary weights loaded 4x
                # per tile instead of 16x; accumulation groups interleave
                # across banks (start q=2f, stop q=2f+1)
                for q in range(4):
                    f, j = q // 2, q % 2
                    for cc in range(NCH):
                        nc.tensor.matmul(
                            out=ps[4 * f + cc][0:nm],
                            lhsT=w_ap[q][0:npart, 0:nm],
                            rhs=xv[0:npart, j, CHUNK * cc:CHUNK * (cc + 1)],
                            start=(j == 0), stop=(j == 1),
                            skip_group_check=True)
                    if j == 1:
                        # filter f complete: drain its 4 banks (contiguous
                        # fp32 -> bf16 casts into the planar output half)
                        for cc in range(NCH):
                            eng = cp_eng[4 * f + cc]
                            out_ap = ov[0:nm, f, CHUNK * cc:CHUNK * (cc + 1)]
                            in_ap = ps[4 * f + cc][0:nm]
                            if eng is nc.scalar:
                                eng.copy(out_ap, in_ap)
                            else:
                                eng.tensor_copy(out=out_ap, in_=in_ap)
                sw = min(SW_ROWS, nm)
                nc.gpsimd.dma_start(out=o_d[M * k:M * k + sw], in_=ot[0:sw])
                if nm > sw:
                    nc.sync.dma_start(out=o_d[M * k + sw:M * k + nm],
                                      in_=ot[sw:nm])

    nc.compile()
    return nc


def _build_w(dec_lo: np.ndarray, dec_hi: np.ndarray) -> np.ndarray:
    """Banded stationary matrices [128, 4*M] bf16: order lo_j0, lo_j1, hi_j0, hi_j1."""
    lo = np.asarray(dec_lo, np.float32)
    hi = np.asarray(dec_hi, np.float32)
    w = np.zeros((128, 4, M), np.float32)
    for m in range(M):
        for d in range(4):
            w[m + d, 0, m] = lo[2 * d]
            w[m + d, 1, m] = lo[2 * d + 1]
            w[m + d, 2, m] = hi[2 * d]
            w[m + d, 3, m] = hi[2 * d + 1]
    return w.reshape(128, 4 * M).astype(NP_BF16)


_NC_CACHE = {}


def _get_nc():
    if "nc" not in _NC_CACHE:
        _NC_CACHE["nc"] = _build_nc()
    return _NC_CACHE["nc"]


def prepare_in_maps(x, dec_lo, dec_hi):
    xb = np.asarray(x).astype(NP_BF16)          # [B, T, C]
    head = xb[:, 1:2]
    tails = [xb[:, T - 2 - j:T - 1 - j] for j in range(5)]
    z = np.zeros((B, 2, C), NP_BF16)
    xe = np.concatenate([head, xb] + tails + [z], axis=1)   # [B, T+8, C]
    w = _build_w(dec_lo, dec_hi)
    return [
        {"x": np.ascontiguousarray(
            xe[:, 2048 * i:2048 * i + SLAB].transpose(1, 0, 2)),
         "w": w}
        for i in range(N_CORES)
    ]


def kernel(x: np.ndarray, dec_lo: np.ndarray, dec_hi: np.ndarray) -> np.ndarray:
    x = np.asarray(x, np.float32)
    assert x.shape == (B, T, C), x.shape
    nc = _get_nc()
    in_maps = prepare_in_maps(x, dec_lo, dec_hi)
    res = run_bass_kernel_spmd(nc, in_maps, core_ids=list(range(N_CORES)))
    # device outputs are [TS, (f b c)] t'-chunks -> concat, crop, interleave
    # to [B, T', 2C] (approx channels then detail channels)
    out = np.concatenate(
        [res.results[i]["out"].reshape(TS, 2, B, C) for i in range(N_CORES)],
        axis=0)
    out = out[:T // 2 - 2].astype(np.float32)        # [T', 2, B, C]
    return np.ascontiguousarray(
        out.transpose(2, 0, 1, 3).reshape(B, T // 2 - 2, 2 * C))
